# revision 19
# baseline (speedup 1.0000x reference)
"""AutomaticBrightnessAndContrast Trainium2 kernel (8-core SPMD).

Algorithm (per core, H-sharded):
  Phase 1: stream image shard, compute gray value, bin index q in [0,256),
           split into hi/lo nibbles, build 16-wide one-hot masks (bin-major
           layout) and accumulate the 16x16 joint histogram on the
           TensorEngine as sum_k onehot16(hi_k) (x) onehot16(lo_k) in PSUM.
           Also track the shard max for the is-normalized check.
  AllReduce (hist 16x16 + not-norm flag) across the 8 cores.
  Scalar section (on-device, replicated): cumulative histogram, min_gray /
           max_gray threshold counts, alpha/beta via exact 255/span lookup
           tables, branchless "unchanged" fallback.
  Phase 2: stream shard again, out = min(relu(x*alpha_eff + beta_eff), hi).

The kernel assumes the normalized-input path (image.max() <= 1.0), which it
verifies on device; if the input is not normalized it falls back to an exact
numpy replica of the reference on host (never taken for uniform [0,1) data).
"""

import numpy as np

P = 128
NB = 16  # nibble bins
T23 = float(2.0 ** 23)
T19 = float(2.0 ** 19)
BIG = 1.0e30

# fp32-exact folded constants
_F = np.float32
C0 = float(_F(255.0) * _F(0.299))
C1 = float(_F(255.0) * _F(0.587))
C2 = float(_F(255.0) * _F(0.114))
INV_BINW = float(_F(1.0) / (_F(255.0) / _F(256.0)))
INV255 = float(_F(1.0) / _F(255.0))

_BUILT = {}


def _alpha_tables():
    s = np.arange(256)
    s_safe = np.where(s == 0, 1, s).astype(np.float32)
    ta = (np.float32(255.0) / s_safe).astype(np.float32)
    tae = (ta / np.float32(255.0)).astype(np.float32)
    return ta.reshape(16, 16), tae.reshape(16, 16)


def _build(free, n_cores, tile_f):
    """Build the Bass program for shards of [3, P, free] per core."""
    from contextlib import ExitStack
    import concourse.bacc as bacc
    import concourse.tile as tile
    from concourse import mybir, bass_isa

    nt = free // tile_f
    npairs = tile_f // 8  # ldweights+matmul pairs per tile

    nc = bacc.Bacc("TRN2", target_bir_lowering=False, debug=False,
                   num_devices=n_cores)
    dt = mybir.dt
    op = mybir.AluOpType
    act = mybir.ActivationFunctionType

    x = nc.dram_tensor("x", [3, P, free], dt.float32, kind="ExternalInput").ap()
    out = nc.dram_tensor("out", [3, P, free], dt.float32,
                         kind="ExternalOutput").ap()
    flag = nc.dram_tensor("flag", [1, 1], dt.float32,
                          kind="ExternalOutput").ap()
    cc_in_t = nc.dram_tensor("cc_in", [17, 16], dt.float32, kind="Internal")
    cc_out_t = nc.dram_tensor("cc_out", [17, 16], dt.float32, kind="Internal",
                              addr_space="Shared")

    # constants
    import ml_dtypes
    # one-hot layout: column j*128 + b*8 + g  <->  (8-pixel group j, bin b,
    # pixel g); each 128-col block is one matmul operand.
    iota_big_np = np.broadcast_to(
        np.tile(np.repeat(np.arange(NB), 8), tile_f // 8).astype(np.float32),
        (P, NB * tile_f))
    iota_big_c = nc.inline_tensor(
        iota_big_np.astype(ml_dtypes.bfloat16), name="iota_big")
    # diag-extract helpers: psum[(b,s),(b',s')] -> hist2d[b,b']
    mask_diag_np = (np.arange(P)[:, None] % 8 ==
                    np.arange(P)[None, :] % 8).astype(np.float32)
    mask_diag_c = nc.inline_tensor(mask_diag_np, name="mask_diag")
    repeye_np = (np.arange(P)[:, None] // 8 ==
                 np.arange(NB)[None, :]).astype(np.float32)
    repeye_c = nc.inline_tensor(repeye_np, name="repeye")
    tri_np = (np.arange(16)[:, None] < np.arange(16)[None, :]).astype(np.float32)
    tri_c = nc.inline_tensor(tri_np, name="tri16")
    iota256_np = (np.arange(256).astype(np.float32)).reshape(16, 16)
    iota256_c = nc.inline_tensor(iota256_np, name="iota256")
    ta_np, tae_np = _alpha_tables()
    ta_c = nc.inline_tensor(ta_np, name="tbl_alpha")
    tae_c = nc.inline_tensor(tae_np, name="tbl_aeff")
    ones16_c = nc.inline_tensor(np.ones((16, 16), np.float32), name="ones16")
    zeros16_c = nc.inline_tensor(np.zeros((16, 16), np.float32), name="zeros16")
    bias_np = np.broadcast_to(np.array(
        [-0.5, T23, -T19, -(15.0 / 32.0), -T23], np.float32), (P, 5))
    bias_c = nc.inline_tensor(np.ascontiguousarray(bias_np), name="biases")

    with tile.TileContext(nc) as tc, ExitStack() as ctx:
        cpool = ctx.enter_context(tc.tile_pool(name="consts", bufs=1))
        work = ctx.enter_context(tc.tile_pool(name="work", bufs=2))
        oh = ctx.enter_context(tc.tile_pool(name="onehot", bufs=2))
        small = ctx.enter_context(tc.tile_pool(name="small", bufs=1))

        # load constants
        iota_big = cpool.tile([P, NB * tile_f], dt.bfloat16)
        nc.sync.dma_start(iota_big[:], iota_big_c.ap())
        mask_diag = cpool.tile([P, P], dt.float32)
        nc.sync.dma_start(mask_diag[:], mask_diag_c.ap())
        repeye = cpool.tile([P, NB], dt.float32)
        nc.sync.dma_start(repeye[:], repeye_c.ap())
        tri16 = cpool.tile([16, 16], dt.float32)
        nc.sync.dma_start(tri16[:], tri_c.ap())
        iota256 = cpool.tile([16, 16], dt.float32)
        nc.sync.dma_start(iota256[:], iota256_c.ap())
        tblA = cpool.tile([16, 16], dt.float32)
        nc.sync.dma_start(tblA[:], ta_c.ap())
        tblAe = cpool.tile([16, 16], dt.float32)
        nc.sync.dma_start(tblAe[:], tae_c.ap())
        ones16 = cpool.tile([16, 16], dt.float32)
        nc.sync.dma_start(ones16[:], ones16_c.ap())
        zeros16 = cpool.tile([16, 16], dt.float32)
        nc.sync.dma_start(zeros16[:], zeros16_c.ap())
        biases = cpool.tile([P, 5], dt.float32)
        nc.sync.dma_start(biases[:], bias_c.ap())
        b_half, b_t23, b_t19, b_1532, b_nt23 = (
            biases[:, i:i + 1] for i in range(5))

        gmax_cols = small.tile([P, 3 * nt], dt.float32)

        with tc.tile_pool(name="jpsum_pool", bufs=1, space="PSUM") as jpool:
            jp = jpool.tile([P, P], dt.float32)

            # ---------------- Phase 1 ----------------
            for t in range(nt):
                sl = slice(t * tile_f, (t + 1) * tile_f)
                xs = []
                for c in range(3):
                    xt = work.tile([P, tile_f], dt.float32, tag=f"x{c}")
                    nc.sync.dma_start(xt[:], x[c, :, sl])
                    xs.append(xt)
                m0 = work.tile([P, tile_f], dt.float32, tag="m0")
                nc.scalar.activation(m0[:], xs[0][:], act.Copy, bias=0.0,
                                     scale=C0)
                s01 = work.tile([P, tile_f], dt.float32, tag="s01")
                nc.vector.scalar_tensor_tensor(s01[:], xs[1][:], C1, m0[:],
                                               op0=op.mult, op1=op.add)
                gray = work.tile([P, tile_f], dt.float32, tag="gray")
                nc.vector.scalar_tensor_tensor(gray[:], xs[2][:], C2, s01[:],
                                               op0=op.mult, op1=op.add)
                qp = work.tile([P, tile_f], dt.float32, tag="qp")
                nc.scalar.activation(qp[:], gray[:], act.Identity, bias=b_half,
                                     scale=INV_BINW)
                zf = work.tile([P, tile_f], dt.float32, tag="zf")
                nc.scalar.activation(zf[:], qp[:], act.Identity, bias=b_t23,
                                     scale=1.0)
                q16 = work.tile([P, tile_f], dt.float32, tag="q16")
                nc.scalar.activation(q16[:], zf[:], act.Identity, bias=b_t19,
                                     scale=1.0 / 16.0)
                yfp = work.tile([P, tile_f], dt.float32, tag="yfp")
                nc.scalar.activation(yfp[:], q16[:], act.Identity,
                                     bias=b_1532, scale=1.0)
                yf = work.tile([P, tile_f], dt.float32, tag="yf")
                nc.scalar.activation(yf[:], yfp[:], act.Identity, bias=b_t23,
                                     scale=1.0)
                hi_b = work.tile([P, tile_f], dt.bfloat16, tag="hi_b")
                nc.scalar.activation(hi_b[:], yf[:], act.Identity, bias=b_nt23,
                                     scale=1.0)
                lo_enc = work.tile([P, tile_f], dt.float32, tag="lo_enc")
                nc.vector.scalar_tensor_tensor(lo_enc[:], hi_b[:], -16.0,
                                               zf[:], op0=op.mult, op1=op.add)
                lo_b = work.tile([P, tile_f], dt.bfloat16, tag="lo_b")
                nc.scalar.activation(lo_b[:], lo_enc[:], act.Identity,
                                     bias=b_nt23, scale=1.0)

                # one-hot masks, j-blocked layout [P, (j, b, g8)]
                Ht = oh.tile([P, NB * tile_f], dt.bfloat16, tag="H")
                Lt = oh.tile([P, NB * tile_f], dt.bfloat16, tag="L")
                iota4 = iota_big[:].rearrange("p (j b g) -> p j b g", b=NB,
                                              g=8)
                hi4 = hi_b[:].rearrange("p (j o g) -> p j o g", o=1,
                                        g=8).broadcast_to(
                    [P, tile_f // 8, NB, 8])
                lo4 = lo_b[:].rearrange("p (j o g) -> p j o g", o=1,
                                        g=8).broadcast_to(
                    [P, tile_f // 8, NB, 8])
                nc.vector.tensor_tensor(
                    Ht[:].rearrange("p (j b g) -> p j b g", b=NB, g=8), hi4,
                    iota4, op.is_equal)
                nc.vector.tensor_tensor(
                    Lt[:].rearrange("p (j b g) -> p j b g", b=NB, g=8), lo4,
                    iota4, op.is_equal)

                # shard max (for is_norm)
                for c in range(3):
                    nc.vector.tensor_reduce(
                        gmax_cols[:, 3 * t + c: 3 * t + c + 1], xs[c][:],
                        axis=mybir.AxisListType.X, op=op.max)

                # joint histogram accumulation on PE
                for j in range(npairs):
                    nc.tensor.matmul(
                        jp[:],
                        Ht[:, P * j: P * j + P],
                        Lt[:, P * j: P * j + P],
                        start=(t == 0 and j == 0),
                        stop=(t == nt - 1 and j == npairs - 1),
                    )

            # ---------------- Phase 1 epilogue ----------------
            # psum[(b,s),(b',s')] -> keep s==s' -> sum over s
            jsb = small.tile([P, P], dt.float32)
            nc.vector.tensor_mul(jsb[:], jp[:], mask_diag[:])

        red = small.tile([P, NB], dt.float32)
        nc.vector.tensor_reduce(red[:],
                                jsb[:].rearrange("p (b g) -> p b g", g=8),
                                axis=mybir.AxisListType.X, op=op.add)
        with tc.tile_pool(name="h2pool", bufs=1, space="PSUM") as hpool:
            h2p = hpool.tile([16, 16], dt.float32)
            nc.tensor.matmul(h2p[:], repeye[:], red[:], start=True, stop=True)
            hist2d = small.tile([16, 16], dt.float32)
            nc.vector.tensor_copy(hist2d[:], h2p[:])

        gm = small.tile([P, 1], dt.float32)
        nc.vector.tensor_reduce(gm[:], gmax_cols[:],
                                axis=mybir.AxisListType.X, op=op.max)
        gma = small.tile([P, 1], dt.float32)
        nc.gpsimd.partition_all_reduce(gma[:], gm[:], channels=P,
                                       reduce_op=bass_isa.ReduceOp.max)
        flg = small.tile([1, 16], dt.float32)
        nc.vector.memset(flg[:], 0.0)
        nc.vector.tensor_single_scalar(flg[:, 0:1], gma[0:1, :], 1.0, op.is_gt)

        cc_in = cc_in_t.ap()
        cc_out = cc_out_t.ap()
        nc.sync.dma_start(cc_in[0:16, :], hist2d[:])
        nc.sync.dma_start(cc_in[16:17, :], flg[:])
        nc.gpsimd.collective_compute(
            "AllReduce", op.add,
            replica_groups=[list(range(n_cores))],
            ins=[cc_in.opt()], outs=[cc_out.opt()],
        )
        hist_g = small.tile([16, 16], dt.float32)
        nc.sync.dma_start(hist_g[:], cc_out[0:16, :])
        nc.sync.dma_start(flag[:], cc_out[16:17, 0:1])

        # ---------------- scalar section ----------------
        rowcum = small.tile([16, 16], dt.float32)
        nc.vector.tensor_tensor_scan(rowcum[:], hist_g[:], zeros16[:], 0.0,
                                     op0=op.add, op1=op.add)
        hsum = small.tile([16, 1], dt.float32)
        nc.vector.tensor_reduce(hsum[:], hist_g[:],
                                axis=mybir.AxisListType.X, op=op.add)
        msum = small.tile([16, 1], dt.float32)
        nc.gpsimd.partition_all_reduce(msum[:], hsum[:], channels=16,
                                       reduce_op=bass_isa.ReduceOp.add)
        with tc.tile_pool(name="ppsum_pool", bufs=1, space="PSUM") as ppool:
            pp = ppool.tile([16, 16], dt.float32)
            nc.tensor.matmul(pp[:, 0:1], tri16[:], hsum[:], start=True,
                             stop=True)
            accm = small.tile([16, 16], dt.float32)
            nc.vector.tensor_single_scalar(accm[:], rowcum[:], pp[:, 0:1],
                                           op.add)
        cv = small.tile([16, 1], dt.float32)
        nc.vector.tensor_single_scalar(cv[:], msum[:], 0.005, op.mult)
        mcv = small.tile([16, 1], dt.float32)
        nc.vector.tensor_sub(mcv[:], msum[:], cv[:])
        cl = small.tile([16, 1], dt.float32)
        clo = small.tile([16, 16], dt.float32, tag="clo")
        nc.vector.scalar_tensor_tensor(clo[:], accm[:], cv[:], ones16[:],
                                       op0=op.is_lt, op1=op.mult,
                                       accum_out=cl[:])
        ch = small.tile([16, 1], dt.float32)
        cho = small.tile([16, 16], dt.float32, tag="cho")
        nc.vector.scalar_tensor_tensor(cho[:], accm[:], mcv[:], ones16[:],
                                       op0=op.is_lt, op1=op.mult,
                                       accum_out=ch[:])
        min_g = small.tile([16, 1], dt.float32)
        nc.gpsimd.partition_all_reduce(min_g[:], cl[:], channels=16,
                                       reduce_op=bass_isa.ReduceOp.add)
        sh = small.tile([16, 1], dt.float32)
        nc.gpsimd.partition_all_reduce(sh[:], ch[:], channels=16,
                                       reduce_op=bass_isa.ReduceOp.add)
        max_g = small.tile([16, 1], dt.float32)
        nc.vector.tensor_single_scalar(max_g[:], sh[:], -1.0, op.add)
        spd = small.tile([16, 1], dt.float32)
        nc.vector.tensor_sub(spd[:], max_g[:], min_g[:])
        span = small.tile([16, 1], dt.float32)
        nc.vector.tensor_single_scalar(span[:], spd[:], 1.0, op.max)
        pred = small.tile([16, 1], dt.float32)
        nc.vector.tensor_tensor(pred[:], max_g[:], min_g[:], op.is_gt)
        mask = small.tile([16, 16], dt.float32)
        nc.vector.tensor_single_scalar(mask[:], iota256[:], span[:],
                                       op.is_equal)
        asel = small.tile([16, 16], dt.float32)
        nc.vector.tensor_mul(asel[:], mask[:], tblA[:])
        ar = small.tile([16, 1], dt.float32)
        nc.vector.tensor_reduce(ar[:], asel[:], axis=mybir.AxisListType.X,
                                op=op.add)
        alpha = small.tile([16, 1], dt.float32)
        nc.gpsimd.partition_all_reduce(alpha[:], ar[:], channels=16,
                                       reduce_op=bass_isa.ReduceOp.add)
        aesel = small.tile([16, 16], dt.float32)
        nc.vector.tensor_mul(aesel[:], mask[:], tblAe[:])
        aer = small.tile([16, 1], dt.float32)
        nc.vector.tensor_reduce(aer[:], aesel[:], axis=mybir.AxisListType.X,
                                op=op.add)
        aeff0 = small.tile([16, 1], dt.float32)
        nc.gpsimd.partition_all_reduce(aeff0[:], aer[:], channels=16,
                                       reduce_op=bass_isa.ReduceOp.add)
        negmin = small.tile([16, 1], dt.float32)
        nc.vector.tensor_single_scalar(negmin[:], min_g[:], -1.0, op.mult)
        beta = small.tile([16, 1], dt.float32)
        nc.vector.tensor_mul(beta[:], negmin[:], alpha[:])
        beff0 = small.tile([16, 1], dt.float32)
        nc.vector.tensor_single_scalar(beff0[:], beta[:], INV255, op.mult)
        # branchless where(max_gray > min_gray)
        am1 = small.tile([16, 1], dt.float32)
        nc.vector.tensor_single_scalar(am1[:], aeff0[:], -1.0, op.add)
        am2 = small.tile([16, 1], dt.float32)
        nc.vector.tensor_mul(am2[:], pred[:], am1[:])
        aeff = small.tile([16, 1], dt.float32)
        nc.vector.tensor_single_scalar(aeff[:], am2[:], 1.0, op.add)
        beff = small.tile([16, 1], dt.float32)
        nc.vector.tensor_mul(beff[:], pred[:], beff0[:])
        hm = small.tile([16, 1], dt.float32)
        nc.vector.tensor_single_scalar(hm[:], pred[:], -1.0, op.add)
        hmb = small.tile([16, 1], dt.float32)
        nc.vector.tensor_single_scalar(hmb[:], hm[:], -BIG, op.mult)
        hic = small.tile([16, 1], dt.float32)
        nc.vector.tensor_add(hic[:], hmb[:], pred[:])

        prow = small.tile([1, 3], dt.float32)
        nc.vector.tensor_copy(prow[:, 0:1], aeff[0:1, :])
        nc.vector.tensor_copy(prow[:, 1:2], beff[0:1, :])
        nc.vector.tensor_copy(prow[:, 2:3], hic[0:1, :])
        par = small.tile([P, 3], dt.float32)
        nc.gpsimd.partition_broadcast(par[:], prow[:], channels=P)

        # ---------------- Phase 2 ----------------
        for c in range(3):
            for t in range(nt):
                sl = slice(t * tile_f, (t + 1) * tile_f)
                xt = work.tile([P, tile_f], dt.float32, tag="p2x")
                nc.sync.dma_start(xt[:], x[c, :, sl])
                r1 = work.tile([P, tile_f], dt.float32, tag="p2r")
                nc.scalar.activation(r1[:], xt[:], act.Relu,
                                     bias=par[:, 1:2], scale=par[:, 0:1])
                r2 = work.tile([P, tile_f], dt.float32, tag="p2o")
                nc.vector.tensor_single_scalar(r2[:], r1[:], par[:, 2:3],
                                               op.min)
                nc.sync.dma_start(out[c, :, sl], r2[:])

    nc.compile()
    return nc


def _numpy_reference(image):
    """Exact numpy replica of the jax reference (host fallback)."""
    f = np.float32
    is_norm = image.max() <= 1.0
    scale = f(255.0) if is_norm else f(1.0)
    imgh = (image * scale).astype(np.float32)
    gray = (f(0.299) * imgh[0] + f(0.587) * imgh[1]) + f(0.114) * imgh[2]
    g = gray.ravel().astype(np.float32)
    bin_w = f(255.0) / f(256.0)
    idx = np.clip(np.floor(g / bin_w), 0, 255).astype(np.int32)
    valid = (g >= 0.0) & (g <= 255.0)
    hist = np.bincount(idx, weights=valid.astype(np.float32),
                       minlength=256).astype(np.float32)
    acc = np.cumsum(hist, dtype=np.float32)
    maximum = acc[-1]
    clip_value = f(1.0) * (maximum / f(100.0)) / f(2.0)
    min_gray = int((acc < clip_value).sum())
    max_gray = int((acc < (maximum - clip_value)).sum()) - 1
    span = np.maximum(f(max_gray - min_gray), f(1.0))
    alpha = f(255.0) / span
    beta = -f(min_gray) * alpha
    alpha_eff = alpha / scale
    beta_eff = beta / scale
    hi = f(1.0) if is_norm else f(255.0)
    adjusted = np.clip(image * alpha_eff + beta_eff, f(0.0), hi)
    return adjusted.astype(np.float32) if max_gray > min_gray else image


def _make_runner(nc, n_cores):
    """Cached jitted shard_map runner (mirrors bass2jax.run_bass_via_pjrt,
    but the compiled executable is reused across calls)."""
    import jax
    from jax.experimental.shard_map import shard_map
    from jax.sharding import Mesh, PartitionSpec
    from concourse import bass2jax, mybir

    bass2jax.install_neuronx_cc_hook()
    partition_name = (nc.partition_id_tensor.name
                      if nc.partition_id_tensor else None)
    in_names, out_names, out_avals = [], [], []
    for alloc in nc.m.functions[0].allocations:
        if not isinstance(alloc, mybir.MemoryLocationSet):
            continue
        name = alloc.memorylocations[0].name
        if alloc.kind == "ExternalInput":
            if name != partition_name:
                in_names.append(name)
        elif alloc.kind == "ExternalOutput":
            out_names.append(name)
            out_avals.append(jax.core.ShapedArray(
                tuple(alloc.tensor_shape), mybir.dt.np(alloc.dtype)))
    n_params = len(in_names)
    all_in = in_names + out_names
    if partition_name is not None:
        all_in.append(partition_name)
    donate = tuple(range(n_params, n_params + len(out_names)))

    def _body(*args):
        operands = list(args)
        if partition_name is not None:
            operands.append(bass2jax.partition_id_tensor())
        return tuple(bass2jax._bass_exec_p.bind(
            *operands,
            out_avals=tuple(out_avals),
            in_names=tuple(all_in),
            out_names=tuple(out_names),
            lowering_input_output_aliases=(),
            sim_require_finite=True,
            sim_require_nnan=True,
            nc=nc,
        ))

    devices = jax.devices()[:n_cores]
    mesh = Mesh(np.asarray(devices), ("core",))
    in_specs = (PartitionSpec("core"),) * (n_params + len(out_names))
    out_specs = (PartitionSpec("core"),) * len(out_names)
    sharded = jax.jit(
        shard_map(_body, mesh=mesh, in_specs=in_specs, out_specs=out_specs,
                  check_rep=False),
        donate_argnums=donate, keep_unused=True)

    out_shapes = [tuple(a.shape) for a in out_avals]
    out_dtypes = [a.dtype for a in out_avals]

    def run(concat_inputs):
        zeros = [np.zeros((n_cores * s[0], *s[1:]), d)
                 for s, d in zip(out_shapes, out_dtypes)]
        outs = sharded(*concat_inputs, *zeros)
        return {name: np.asarray(outs[i]).reshape(n_cores, *out_shapes[i])
                for i, name in enumerate(out_names)}

    run.sharded = sharded
    run.n_params = n_params
    run.out_shapes = out_shapes
    run.out_dtypes = out_dtypes
    run.n_cores = n_cores
    return run


def _get_runner(free, n_cores, tile_f=512):
    key = (free, n_cores, tile_f)
    if key not in _BUILT:
        nc = _build(free, n_cores, tile_f=tile_f)
        _BUILT[key] = _make_runner(nc, n_cores)
    return _BUILT[key]


def kernel(image):
    image = np.ascontiguousarray(np.asarray(image, dtype=np.float32))
    assert image.shape == (3, 4096, 4096), image.shape

    n_cores = 8
    rows = image.shape[1] // n_cores          # 512
    free = rows * image.shape[2] // P         # 16384
    run = _get_runner(free, n_cores)

    # concat per-core shards along axis 0: [3*n_cores, P, free]
    x_all = image.reshape(3, n_cores, P, free).transpose(1, 0, 2, 3) \
                 .reshape(n_cores * 3, P, free)
    res = run([np.ascontiguousarray(x_all)])
    if float(res["flag"].max()) > 0.0:
        return _numpy_reference(image)

    # res["out"]: [n_cores, 3, P, free] -> [3, 4096, 4096]
    out = res["out"].transpose(1, 0, 2, 3).reshape(3, 4096, 4096)
    return np.ascontiguousarray(out)
